# revision 11
# baseline (speedup 1.0000x reference)
"""Trainium2 Bass kernel for nn_DynamicMaxSimilarity — anti-diagonal DP.

Full inputs a,b: [512, 16, 256] f32.
  an = l2norm(tanh(a)) rows; bn likewise
  sim[a,b,i,j] = dot(an[a,i], bn[b,j]);  out[a,b] = DTW-like max-avg DP:
  si[i,j] = (max(si[i-1,j-1], si[i-1,j], si[i,j-1])*(m-1) + sim[i,j])/m,
  m = max(i,j), zero borders; answer si[16,16].

Sharding: 8 cores as 4 a-chunks (128) x 2 b-chunks (256). Per-core block
[128 a, 256 b]; pairs live as [128 partitions (a), 256 free (b)].

Design (vs the 201us L-border/scan baseline): process cells (i,j) by
anti-diagonal d=i+j in the *si domain*, which kills the per-slot
coefficient scaling and the 1.08ns/elem scan/STT ops entirely:
- state si kept as fp16 SBUF tiles U_d [128, 18*256] (phys slot = i,
  zero guard slots; 3 rotating buffers zero-initialized once).
- per diag: max1 = TT(U_{d-1}[i-1], U_{d-1}[i]); max2 = TT(max1,
  U_{d-2}[i-1]) — plain fp16 TTs run at 0.56 ns/elem (2x_1p), the only
  DVE work per cell.
- psum plane for (i,j) accumulates sim/m directly by pre-scaling the
  matmul operands (upper j>i: aT[i]*bTs[j], else aTs[i]*bT[j], where
  aTs = an/i, bTs = bn/j), then PE adds best*(m-1)/m via a diagonal
  weight matmul W=((m-1)/m)*I (contraction rows are free in PE cost).
  psum then holds si[i,j] exactly; eviction is a plain batched copy.
- PSUM accumulation groups are PER BANK (a start=True matmul into the
  other half of a bank kills the open group), so each plane gets a full
  2KB bank: ring of 8 banks, cell n -> bank n mod 8; a bank's chain
  [sim kh0 (start), sim kh1, accum (stop)] for cell n fully precedes
  cell n+8's chain in PE program order (sims pumped in global cell
  order, gated on the eviction of cell n-8 and on operand frames).
- evictions: lead pieces on Pool (gpsimd tensor_scalar from PSUM),
  tail pieces on ACT — splits the elems and shortens the diag chain.
- loads/normalize interleaved per 4-frame quarter with early DP diags
  so the DP starts ~12us in instead of ~40us.
"""

import numpy as np

import concourse.bass as bass
from concourse import bacc
import concourse.mybir as mybir
from concourse.tile import TileContext
from concourse import bass_utils

NA, NB, T, D = 512, 512, 16, 256
ACH, BCH = 128, 256
P = 128
F = BCH              # psum cols per cell plane
KH = D // 128
DT = mybir.dt.float32
HT = mybir.dt.float16
IT = mybir.dt.int16
ALU = mybir.AluOpType
ACTF = mybir.ActivationFunctionType

_last_results = None


def _cells(d):
    i0, i1 = max(1, d - 16), min(16, d - 1)
    return list(range(i0, i1 + 1))


def _pieces(cells):
    """Split a diag's cells into pieces: small leading pieces cut the
    diag-to-diag latency chain; the tail amortizes overhead. Piece k
    covering global offsets [o0, o1) must satisfy o1 <= (prefix before
    it) + 8 so same-diag bank reuse stays ordered (sizes [2,2,4,8]:
    prefix 8 before the [8,16) piece — exactly the ring constraint)."""
    out = []
    i = 0
    sizes = [2, 2, 4, 8]
    k = 0
    while i < len(cells):
        w = sizes[k] if k < len(sizes) else 8
        out.append(cells[i:i + w])
        i += w
        k += 1
    return out


def build_program():
    nc = bacc.Bacc("TRN2", target_bir_lowering=False, debug=False)

    a_d = nc.dram_tensor("a_c", [ACH, T, D], DT, kind="ExternalInput")
    b_d = nc.dram_tensor("b_c", [BCH, T, D], DT, kind="ExternalInput")
    out_d = nc.dram_tensor("out", [ACH, BCH], DT, kind="ExternalOutput")

    with TileContext(nc) as tc:
        with (
            tc.tile_pool(name="mp", bufs=1) as mp,
            tc.tile_pool(name="wp", bufs=2) as wp,
            tc.tile_pool(name="pp", bufs=1, space="PSUM") as pp,
        ):
            # ---- loads: quarters interleaved a/b so early frames of all
            # blocks land first ----
            a_sb = mp.tile([P, T, D], DT, tag="ld_a")
            b_sb = [mp.tile([P, T, D], DT, name=f"b_sb{h}", tag=f"ld_b{h}")
                    for h in range(2)]
            for q in range(4):
                sl = slice(q * 4, (q + 1) * 4)
                nc.sync.dma_start(a_sb[:, sl, :], a_d.ap()[:, sl, :])
                for h in range(2):
                    nc.sync.dma_start(
                        b_sb[h][:, sl, :],
                        b_d.ap()[h * 128:(h + 1) * 128, sl, :])

            # ---- diagonal weight tiles W[m] = ((m-1)/m) * I_128, fp16 ----
            iota_t = mp.tile([P, 128], IT)
            nc.gpsimd.iota(iota_t[:, :], pattern=[[1, 128]], base=0,
                           channel_multiplier=-1)
            ident = mp.tile([P, 128], HT)
            nc.vector.tensor_scalar(ident[:, :], iota_t[:, :], 0, None,
                                    ALU.is_equal)
            wm = mp.tile([P, 16, 128], HT)
            for m in range(2, 17):
                nc.vector.tensor_scalar(wm[:, m - 1, :], ident[:, :],
                                        float((m - 1) / m), None, ALU.mult)

            # ---- DP state: si diag buffers, 18 slots (idx 0/17 guards) ----
            U = [mp.tile([P, 18 * F], HT, name=f"U{x}") for x in range(3)]
            for x in range(3):
                nc.gpsimd.memset(U[x][:, :], 0.0)
            B = [mp.tile([P, 18 * F], HT, name=f"B{x}") for x in range(2)]

            # ---- normalize tiles ----
            ah = mp.tile([P, T, D], HT)
            bh = [mp.tile([P, T, D], HT, name=f"bh{h}") for h in range(2)]
            ssq = mp.tile([P, 3, T], DT)
            nrm = mp.tile([P, 3, T], DT)
            rinv = mp.tile([P, 3, T], DT)
            aT = mp.tile([P, T * KH, P], HT)        # [d, i*2+kh, a]
            aTs = mp.tile([P, T * KH, P], HT)       # scaled by 1/i
            bT = mp.tile([P, T, KH, 2, P], HT)      # [d, j, kh, half, b]
            bTs = mp.tile([P, T, KH, 2, P], HT)     # scaled by 1/j
            blocks = [(a_sb, ah, 0), (b_sb[0], bh[0], 1), (b_sb[1], bh[1], 2)]

            def normalize_quarter(bi, q):
                x_sb, xh, _ = blocks[bi]
                sl = slice(q * 4, (q + 1) * 4)
                nc.scalar.activation(xh[:, sl, :], x_sb[:, sl, :], ACTF.Tanh)
                # sumsq: frame 4q on ACT (Square+accum), rest on DVE
                sqa = wp.tile([P, D], HT, name=f"sqa{bi}_{q}", tag="sqa")
                nc.scalar.activation(
                    sqa[:, :], xh[:, q * 4, :], ACTF.Square,
                    accum_out=ssq[:, bi, q * 4:q * 4 + 1])
                sq = wp.tile([P, 3, D], HT, name=f"sq{bi}_{q}", tag="sq")
                sl3 = slice(q * 4 + 1, (q + 1) * 4)
                nc.gpsimd.tensor_tensor(sq[:, :, :], xh[:, sl3, :],
                                        xh[:, sl3, :], ALU.mult)
                nc.vector.tensor_reduce(ssq[:, bi, sl3], sq[:, :, :],
                                        mybir.AxisListType.X, ALU.add)
                # rinv = rsqrt(ssq) via int bit trick + 1 Newton step
                sv = ssq[:, bi, sl]
                yv = rinv[:, bi, sl]
                wv = nrm[:, bi, sl]
                nc.vector.tensor_scalar(yv.bitcast(mybir.dt.int32),
                                        sv.bitcast(mybir.dt.int32),
                                        1, None, ALU.logical_shift_right)
                nc.vector.tensor_scalar(yv.bitcast(mybir.dt.int32),
                                        yv.bitcast(mybir.dt.int32),
                                        0x5F3759DF, -1,
                                        ALU.subtract, ALU.mult)
                nc.vector.tensor_tensor(wv, yv, yv, ALU.mult)
                nc.vector.tensor_tensor(wv, wv, sv, ALU.mult)
                nc.vector.tensor_scalar(wv, wv, -0.5, 1.5, ALU.mult, ALU.add)
                nc.vector.tensor_tensor(yv, yv, wv, ALU.mult)
                for i in range(q * 4, (q + 1) * 4):
                    nc.vector.tensor_scalar_mul(xh[:, i, :], xh[:, i, :],
                                                rinv[:, bi, i:i + 1])
                # scaled copy: xs[frame f] = xh[f] / (f+1)
                xs = wp.tile([P, 4, D], HT, name=f"xs{bi}_{q}", tag="xs")
                for i in range(q * 4, (q + 1) * 4):
                    nc.vector.tensor_scalar(xs[:, i - q * 4, :], xh[:, i, :],
                                            float(1.0 / (i + 1)), None,
                                            ALU.mult)
                if bi == 0:
                    nc.sync.dma_start_transpose(
                        aT[:, q * 8:(q + 1) * 8, :], xh[:, sl, :])
                    nc.sync.dma_start_transpose(
                        aTs[:, q * 8:(q + 1) * 8, :], xs[:, :, :])
                else:
                    nc.sync.dma_start_transpose(
                        bT[:, sl, :, bi - 1, :], xh[:, sl, :])
                    nc.sync.dma_start_transpose(
                        bTs[:, sl, :, bi - 1, :], xs[:, :, :])

            def amat(i, kh, scaled):
                # frame i is 1-based
                t = aTs if scaled else aT
                return t[:, (i - 1) * KH + kh, :]

            def bmov(j, kh, scaled):
                t = bTs if scaled else bT
                return t[:, j - 1, kh, :, :]

            # ---- DP plumbing ----
            PS = pp.tile([P, 16 * F], DT)   # 8 banks x 512 fp32

            n_of = {}
            cnt = 0
            order = []
            for dd in range(2, 33):
                for ii in _cells(dd):
                    n_of[(dd, ii)] = cnt
                    order.append((dd, ii))
                    cnt += 1

            def bank(d, i):
                # per-diag mapping: diag cells start at bank 0, so diag
                # eviction runs never wrap the ring
                return (i - _cells(d)[0]) % 8

            def pcol(d, i):
                return bank(d, i) * 2 * F

            state = {"sim": 0, "evicted": 0, "max_frame": 0}
            bank_holder = [-1] * 8   # bank -> global n of last sim issued

            def pump_sims():
                # sims in global cell order; cell n waits for its bank's
                # previous holder to have its eviction issued, and for its
                # frames to be normalized
                while state["sim"] < 256:
                    n = state["sim"]
                    d, i = order[n]
                    j = d - i
                    m = max(i, j)
                    if m > state["max_frame"]:
                        return
                    b = bank(d, i)
                    if bank_holder[b] >= state["evicted"]:
                        return
                    bank_holder[b] = n
                    state["sim"] += 1
                    c0 = pcol(d, i)
                    dst = PS[:, c0:c0 + F]
                    for kh in range(KH):
                        nc.tensor.matmul(
                            dst, amat(i, kh, scaled=(i >= j)),
                            bmov(j, kh, scaled=(j > i)),
                            start=(kh == 0),
                            stop=(kh == KH - 1 and m == 1))

            out_sb = mp.tile([P, F], DT)

            def useg(t, a, b):
                return t[:, a * F:(b + 1) * F]

            def dp_step(d):
                cells = _cells(d)
                Ud = U[d % 3]
                U1 = U[(d - 1) % 3]
                U2 = U[(d - 2) % 3]
                Bd = B[d % 2]
                pieces = _pieces(cells)

                # all maxes up front (DVE streams independently)
                if d > 2:
                    for pc in pieces:
                        p0, p1 = pc[0], pc[-1]
                        nc.vector.tensor_tensor(
                            useg(Bd, p0, p1), useg(U1, p0 - 1, p1 - 1),
                            useg(U1, p0, p1), ALU.max)
                        nc.vector.tensor_tensor(
                            useg(Bd, p0, p1), useg(Bd, p0, p1),
                            useg(U2, p0 - 1, p1 - 1), ALU.max)

                if d == 32:
                    i = 16
                    nc.tensor.matmul(
                        PS[:, pcol(32, 16):pcol(32, 16) + F],
                        wm[:, 15, :], Bd[:, i * F:(i + 1) * F],
                        start=False, stop=True)
                    nc.scalar.activation(
                        out_sb[:, :],
                        PS[:, pcol(32, 16):pcol(32, 16) + F], ACTF.Copy)
                    return

                # per piece: accums -> ACT evict -> pump freed sims
                for pc in pieces:
                    if d > 2:
                        for i in pc:
                            m = max(i, d - i)
                            c0 = pcol(d, i)
                            nc.tensor.matmul(
                                PS[:, c0:c0 + F], wm[:, m - 1, :],
                                Bd[:, i * F:(i + 1) * F],
                                start=False, stop=True)
                    ri = pc[0]
                    rem = len(pc)
                    while rem > 0:
                        s = bank(d, ri)
                        w = min(rem, 8 - s)
                        src = PS[:, s * 2 * F:(s + w) * 2 * F].rearrange(
                            "p (c k) -> p c k", k=2 * F)[:, :, 0:F]
                        dst = useg(Ud, ri, ri + w - 1).rearrange(
                            "p (c k) -> p c k", k=F)
                        nc.scalar.activation(dst, src, ACTF.Copy)
                        ri += w
                        rem -= w
                    state["evicted"] += len(pc)
                    pump_sims()

            # ---- interleave normalize quarters with DP diagonals ----
            next_d = 2
            for q in range(4):
                for bi in range(3):
                    normalize_quarter(bi, q)
                state["max_frame"] = 4 * (q + 1)
                pump_sims()
                d_limit = 4 * q + 5 if q < 3 else 32
                while next_d <= d_limit:
                    dp_step(next_d)
                    next_d += 1

            nc.sync.dma_start(out_d.ap(), out_sb[:, :])

    nc.compile()
    return nc


def kernel(a: np.ndarray, b: np.ndarray) -> np.ndarray:
    a = np.ascontiguousarray(a, dtype=np.float32)
    b = np.ascontiguousarray(b, dtype=np.float32)
    assert a.shape == (NA, T, D) and b.shape == (NB, T, D)

    nc = build_program()

    in_maps = []
    for core in range(8):
        ca, cb = core // 2, core % 2
        in_maps.append({
            "a_c": a[ca * ACH:(ca + 1) * ACH],
            "b_c": b[cb * BCH:(cb + 1) * BCH],
        })

    res = bass_utils.run_bass_kernel_spmd(nc, in_maps, core_ids=list(range(8)))
    global _last_results
    _last_results = res

    out = np.zeros((NA, NB), dtype=np.float32)
    for core in range(8):
        ca, cb = core // 2, core % 2
        out[ca * ACH:(ca + 1) * ACH, cb * BCH:(cb + 1) * BCH] = \
            res.results[core]["out"]
    return out


# revision 18
# speedup vs baseline: 1.0353x; 1.0353x over previous
"""Trainium2 Bass kernel for nn_DynamicMaxSimilarity — anti-diagonal DP.

Full inputs a,b: [512, 16, 256] f32.
  an = l2norm(tanh(a)) rows; bn likewise
  sim[a,b,i,j] = dot(an[a,i], bn[b,j]);  out[a,b] = DTW-like max-avg DP:
  si[i,j] = (max(si[i-1,j-1], si[i-1,j], si[i,j-1])*(m-1) + sim[i,j])/m,
  m = max(i,j), zero borders; answer si[16,16].

Sharding: 8 cores as 4 a-chunks (128) x 2 b-chunks (256). Per-core block
[128 a, 256 b]; pairs live as [128 partitions (a), 256 free (b)].

Design (vs the 201us L-border/scan baseline): process cells (i,j) by
anti-diagonal d=i+j in the *si domain*, which kills the per-slot
coefficient scaling and the 1.08ns/elem scan/STT ops entirely:
- state si kept as fp16 SBUF tiles U_d [128, 18*256] (phys slot = i,
  zero guard slots; 3 rotating buffers zero-initialized once).
- per diag: max1 = TT(U_{d-1}[i-1], U_{d-1}[i]); max2 = TT(max1,
  U_{d-2}[i-1]) — plain fp16 TTs run at 0.56 ns/elem (2x_1p), the only
  DVE work per cell.
- psum plane for (i,j) accumulates sim/m directly by pre-scaling the
  matmul operands (upper j>i: aT[i]*bTs[j], else aTs[i]*bT[j], where
  aTs = an/i, bTs = bn/j), then PE adds best*(m-1)/m via a diagonal
  weight matmul W=((m-1)/m)*I (contraction rows are free in PE cost).
  psum then holds si[i,j] exactly; eviction is a plain batched copy.
- PSUM accumulation groups are PER BANK (a start=True matmul into the
  other half of a bank kills the open group), so each plane gets a full
  2KB bank: ring of 8 banks, cell n -> bank n mod 8; a bank's chain
  [sim kh0 (start), sim kh1, accum (stop)] for cell n fully precedes
  cell n+8's chain in PE program order (sims pumped in global cell
  order, gated on the eviction of cell n-8 and on operand frames).
- evictions: lead pieces on Pool (gpsimd tensor_scalar from PSUM),
  tail pieces on ACT — splits the elems and shortens the diag chain.
- loads/normalize interleaved per 4-frame quarter with early DP diags
  so the DP starts ~12us in instead of ~40us.
"""

import numpy as np

import concourse.bass as bass
from concourse import bacc
import concourse.mybir as mybir
from concourse.tile import TileContext
from concourse import bass_utils

NA, NB, T, D = 512, 512, 16, 256
ACH, BCH = 128, 256
P = 128
F = BCH              # psum cols per cell plane
KH = D // 128
DT = mybir.dt.float32
HT = mybir.dt.float16
IT = mybir.dt.int16
ALU = mybir.AluOpType
ACTF = mybir.ActivationFunctionType

_last_results = None


def _cells(d):
    i0, i1 = max(1, d - 16), min(16, d - 1)
    return list(range(i0, i1 + 1))


def _pieces(cells):
    """Split a diag's cells into pieces: small leading pieces cut the
    diag-to-diag latency chain; the tail amortizes overhead. Piece k
    covering global offsets [o0, o1) must satisfy o1 <= (prefix before
    it) + 8 so same-diag bank reuse stays ordered (sizes [2,2,4,8]:
    prefix 8 before the [8,16) piece — exactly the ring constraint)."""
    out = []
    i = 0
    sizes = [2, 2, 4, 8]
    k = 0
    while i < len(cells):
        w = sizes[k] if k < len(sizes) else 8
        out.append(cells[i:i + w])
        i += w
        k += 1
    return out


def build_program():
    nc = bacc.Bacc("TRN2", target_bir_lowering=False, debug=False)

    a_d = nc.dram_tensor("a_c", [ACH, T, D], DT, kind="ExternalInput")
    b_d = nc.dram_tensor("b_c", [BCH, T, D], DT, kind="ExternalInput")
    out_d = nc.dram_tensor("out", [ACH, BCH], DT, kind="ExternalOutput")

    with TileContext(nc) as tc:
        with (
            tc.tile_pool(name="mp", bufs=1) as mp,
            tc.tile_pool(name="wp", bufs=2) as wp,
            tc.tile_pool(name="pp", bufs=1, space="PSUM") as pp,
        ):
            # ---- loads: quarters interleaved a/b so early frames of all
            # blocks land first ----
            a_sb = mp.tile([P, T, D], DT, tag="ld_a")
            b_sb = [mp.tile([P, T, D], DT, name=f"b_sb{h}", tag=f"ld_b{h}")
                    for h in range(2)]
            for q in range(4):
                sl = slice(q * 4, (q + 1) * 4)
                nc.sync.dma_start(a_sb[:, sl, :], a_d.ap()[:, sl, :])
                for h in range(2):
                    nc.sync.dma_start(
                        b_sb[h][:, sl, :],
                        b_d.ap()[h * 128:(h + 1) * 128, sl, :])

            # ---- diagonal weight tiles W[m] = ((m-1)/m) * I_128, fp16 ----
            iota_t = mp.tile([P, 128], IT)
            nc.gpsimd.iota(iota_t[:, :], pattern=[[1, 128]], base=0,
                           channel_multiplier=-1)
            ident = mp.tile([P, 128], HT)
            nc.vector.tensor_scalar(ident[:, :], iota_t[:, :], 0, None,
                                    ALU.is_equal)
            wm = mp.tile([P, 16, 128], HT)
            for m in range(2, 17):
                nc.vector.tensor_scalar(wm[:, m - 1, :], ident[:, :],
                                        float((m - 1) / m), None, ALU.mult)

            # ---- DP state: si diag buffers, 18 slots (idx 0/17 guards) ----
            U = [mp.tile([P, 18 * F], HT, name=f"U{x}") for x in range(3)]
            for x in range(3):
                nc.gpsimd.memset(U[x][:, :], 0.0)
            B = [mp.tile([P, 18 * F], HT, name=f"B{x}") for x in range(2)]

            # ---- normalize tiles ----
            ah = mp.tile([P, T, D], HT)
            bh = [mp.tile([P, T, D], HT, name=f"bh{h}") for h in range(2)]
            ssq = mp.tile([P, 3, T], DT)
            nrm = mp.tile([P, 3, T], DT)
            rinv = mp.tile([P, 3, T], DT)
            aT = mp.tile([P, T * KH, P], HT)        # [d, i*2+kh, a]
            aTs = mp.tile([P, T * KH, P], HT)       # scaled by 1/i
            bT = mp.tile([P, T, KH, 2, P], HT)      # [d, j, kh, half, b]
            bTs = mp.tile([P, T, KH, 2, P], HT)     # scaled by 1/j
            blocks = [(a_sb, ah, 0), (b_sb[0], bh[0], 1), (b_sb[1], bh[1], 2)]

            def normalize_quarter(bi, q):
                x_sb, xh, _ = blocks[bi]
                sl = slice(q * 4, (q + 1) * 4)
                nc.scalar.activation(xh[:, sl, :], x_sb[:, sl, :], ACTF.Tanh)
                # sumsq: frame 4q on ACT (Square+accum), rest on DVE
                sqa = wp.tile([P, D], HT, name=f"sqa{bi}_{q}", tag="sqa")
                nc.scalar.activation(
                    sqa[:, :], xh[:, q * 4, :], ACTF.Square,
                    accum_out=ssq[:, bi, q * 4:q * 4 + 1])
                sq = wp.tile([P, 3, D], HT, name=f"sq{bi}_{q}", tag="sq")
                sl3 = slice(q * 4 + 1, (q + 1) * 4)
                nc.gpsimd.tensor_tensor(sq[:, :, :], xh[:, sl3, :],
                                        xh[:, sl3, :], ALU.mult)
                nc.vector.tensor_reduce(ssq[:, bi, sl3], sq[:, :, :],
                                        mybir.AxisListType.X, ALU.add)
                # rinv = rsqrt(ssq) via int bit trick + 1 Newton step
                sv = ssq[:, bi, sl]
                yv = rinv[:, bi, sl]
                wv = nrm[:, bi, sl]
                nc.vector.tensor_scalar(yv.bitcast(mybir.dt.int32),
                                        sv.bitcast(mybir.dt.int32),
                                        1, None, ALU.logical_shift_right)
                nc.vector.tensor_scalar(yv.bitcast(mybir.dt.int32),
                                        yv.bitcast(mybir.dt.int32),
                                        0x5F3759DF, -1,
                                        ALU.subtract, ALU.mult)
                nc.vector.tensor_tensor(wv, yv, yv, ALU.mult)
                nc.vector.tensor_tensor(wv, wv, sv, ALU.mult)
                nc.vector.tensor_scalar(wv, wv, -0.5, 1.5, ALU.mult, ALU.add)
                nc.vector.tensor_tensor(yv, yv, wv, ALU.mult)
                for i in range(q * 4, (q + 1) * 4):
                    nc.vector.tensor_scalar_mul(xh[:, i, :], xh[:, i, :],
                                                rinv[:, bi, i:i + 1])
                # scaled copy: xs[frame f] = xh[f] / (f+1)
                xs = wp.tile([P, 4, D], HT, name=f"xs{bi}_{q}", tag="xs")
                for i in range(q * 4, (q + 1) * 4):
                    nc.vector.tensor_scalar(xs[:, i - q * 4, :], xh[:, i, :],
                                            float(1.0 / (i + 1)), None,
                                            ALU.mult)
                if bi == 0:
                    nc.sync.dma_start_transpose(
                        aT[:, q * 8:(q + 1) * 8, :], xh[:, sl, :])
                    nc.sync.dma_start_transpose(
                        aTs[:, q * 8:(q + 1) * 8, :], xs[:, :, :])
                else:
                    nc.sync.dma_start_transpose(
                        bT[:, sl, :, bi - 1, :], xh[:, sl, :])
                    nc.sync.dma_start_transpose(
                        bTs[:, sl, :, bi - 1, :], xs[:, :, :])

            def amat(i, kh, scaled):
                # frame i is 1-based
                t = aTs if scaled else aT
                return t[:, (i - 1) * KH + kh, :]

            def bmov(j, kh, scaled):
                t = bTs if scaled else bT
                return t[:, j - 1, kh, :, :]

            # ---- DP plumbing ----
            PS = pp.tile([P, 16 * F], DT)   # 8 banks x 512 fp32

            n_of = {}
            nst = {}
            cnt = 0
            order = []
            for dd in range(2, 33):
                nst[dd] = cnt
                for ii in _cells(dd):
                    n_of[(dd, ii)] = cnt
                    order.append((dd, ii))
                    cnt += 1

            def bank(d, i):
                # per-diag mapping: diag cells start at bank 0, so diag
                # eviction runs never wrap the ring
                return (i - _cells(d)[0]) % 8

            def pcol(d, i):
                return bank(d, i) * 2 * F

            state = {"sim": 0, "evicted": 0, "max_frame": 0}
            bank_holder = [-1] * 8   # bank -> global n of last sim issued

            def pump_upto(n_stop):
                # issue sim matmuls in global cell order for cells n <
                # n_stop; stop early if a gate (bank WAR not yet evicted,
                # frames not normalized) blocks. Keeping n_stop tight
                # prevents sim bursts from clogging the PE queue ahead of
                # chain-critical accum matmuls.
                while state["sim"] < min(n_stop, 256):
                    n = state["sim"]
                    d, i = order[n]
                    j = d - i
                    m = max(i, j)
                    if m > state["max_frame"]:
                        return
                    b = bank(d, i)
                    if bank_holder[b] >= state["evicted"]:
                        return
                    bank_holder[b] = n
                    state["sim"] += 1
                    c0 = pcol(d, i)
                    dst = PS[:, c0:c0 + F]
                    for kh in range(KH):
                        nc.tensor.matmul(
                            dst, amat(i, kh, scaled=(i >= j)),
                            bmov(j, kh, scaled=(j > i)),
                            start=(kh == 0),
                            stop=(kh == KH - 1 and m == 1))

            out_sb = mp.tile([P, F], DT)

            def useg(t, a, b):
                return t[:, a * F:(b + 1) * F]

            def dp_step(d):
                cells = _cells(d)
                Ud = U[d % 3]
                U1 = U[(d - 1) % 3]
                U2 = U[(d - 2) % 3]
                Bd = B[d % 2]
                pieces = _pieces(cells)

                # all maxes up front (DVE streams independently)
                if d > 2:
                    for pc in pieces:
                        p0, p1 = pc[0], pc[-1]
                        nc.vector.tensor_tensor(
                            useg(Bd, p0, p1), useg(U1, p0 - 1, p1 - 1),
                            useg(U1, p0, p1), ALU.max)
                        nc.vector.tensor_tensor(
                            useg(Bd, p0, p1), useg(Bd, p0, p1),
                            useg(U2, p0 - 1, p1 - 1), ALU.max)

                if d == 32:
                    i = 16
                    nc.tensor.matmul(
                        PS[:, pcol(32, 16):pcol(32, 16) + F],
                        wm[:, 15, :], Bd[:, i * F:(i + 1) * F],
                        start=False, stop=True)
                    nc.scalar.activation(
                        out_sb[:, :],
                        PS[:, pcol(32, 16):pcol(32, 16) + F], ACTF.Copy)
                    return

                # per piece: required sims -> accums -> ACT evict
                for pc in pieces:
                    pump_upto(n_of[(d, pc[-1])] + 1)
                    assert state["sim"] > n_of[(d, pc[-1])], (d, pc)
                    if d > 2:
                        for i in pc:
                            m = max(i, d - i)
                            c0 = pcol(d, i)
                            nc.tensor.matmul(
                                PS[:, c0:c0 + F], wm[:, m - 1, :],
                                Bd[:, i * F:(i + 1) * F],
                                start=False, stop=True)
                    ri = pc[0]
                    rem = len(pc)
                    while rem > 0:
                        s = bank(d, ri)
                        w = min(rem, 8 - s)
                        src = PS[:, s * 2 * F:(s + w) * 2 * F].rearrange(
                            "p (c k) -> p c k", k=2 * F)[:, :, 0:F]
                        dst = useg(Ud, ri, ri + w - 1).rearrange(
                            "p (c k) -> p c k", k=F)
                        nc.scalar.activation(dst, src, ACTF.Copy)
                        ri += w
                        rem -= w
                    state["evicted"] += len(pc)
                # small lead into the next diag so PE has work at step start
                if d < 32:
                    pump_upto(nst[d + 1] + 2)

            # ---- interleave normalize quarters with DP diagonals ----
            next_d = 2
            for q in range(4):
                for bi in range(3):
                    normalize_quarter(bi, q)
                state["max_frame"] = 4 * (q + 1)
                pump_upto(nst[min(next_d, 32)] + 2)
                d_limit = 4 * q + 5 if q < 3 else 32
                while next_d <= d_limit:
                    dp_step(next_d)
                    next_d += 1

            nc.sync.dma_start(out_d.ap(), out_sb[:, :])

    nc.compile()
    return nc


def kernel(a: np.ndarray, b: np.ndarray) -> np.ndarray:
    a = np.ascontiguousarray(a, dtype=np.float32)
    b = np.ascontiguousarray(b, dtype=np.float32)
    assert a.shape == (NA, T, D) and b.shape == (NB, T, D)

    nc = build_program()

    in_maps = []
    for core in range(8):
        ca, cb = core // 2, core % 2
        in_maps.append({
            "a_c": a[ca * ACH:(ca + 1) * ACH],
            "b_c": b[cb * BCH:(cb + 1) * BCH],
        })

    res = bass_utils.run_bass_kernel_spmd(nc, in_maps, core_ids=list(range(8)))
    global _last_results
    _last_results = res

    out = np.zeros((NA, NB), dtype=np.float32)
    for core in range(8):
        ca, cb = core // 2, core % 2
        out[ca * ACH:(ca + 1) * ACH, cb * BCH:(cb + 1) * BCH] = \
            res.results[core]["out"]
    return out


# revision 35
# speedup vs baseline: 1.1780x; 1.1378x over previous
"""Trainium2 Bass kernel for nn_DynamicMaxSimilarity — anti-diagonal DP.

Full inputs a,b: [512, 16, 256] f32.
  an = l2norm(tanh(a)) rows; bn likewise
  sim[a,b,i,j] = dot(an[a,i], bn[b,j]);  out[a,b] = DTW-like max-avg DP:
  si[i,j] = (max(si[i-1,j-1], si[i-1,j], si[i,j-1])*(m-1) + sim[i,j])/m,
  m = max(i,j), zero borders; answer si[16,16].

Sharding: 8 cores as 4 a-chunks (128) x 2 b-chunks (256). Per-core block
[128 a, 256 b]; pairs live as [128 partitions (a), 256 free (b)].

Design (vs the 201us L-border/scan baseline): process cells (i,j) by
anti-diagonal d=i+j, killing the per-slot coefficient scaling and the
1.08ns/elem scan/STT ops entirely:
- state si kept as fp16 SBUF tiles U_d [128, 18*256] (phys slot = i,
  zero guard slots; 3 rotating buffers zero-initialized once).
- cells processed ends-inward per diag as transpose PAIRS {(i,j),(j,i)}
  which share m = max(i,j): all per-pair views (maxes, accum moving,
  psum, eviction) are regular 2-range strided APs.
- per diag: max1 = TT(U_{d-1}[·-1], U_{d-1}[·]); max2 = TT(max1,
  U_{d-2}[·-1]) — plain fp16 TTs at 0.56 ns/elem (2x_1p), the only DVE
  work per cell.
- psum plane for (i,j) accumulates u = sim + best*(m-1): sims feed raw
  aT/bT; PE adds best*(m-1) via one diagonal-weight matmul per pair
  (W=(m-1)*I fp16, exact integers; contraction rows are free in PE
  cost). The ACT eviction applies the single shared 1/m scale per pair.
- PSUM accumulation groups are PER BANK, so each plane gets a full 2KB
  bank: cell at in-diag position p -> bank p mod 8; a bank's chain
  [sim kh0 (start), sim kh1, accum (stop)] for one cell fully precedes
  the next holder's chain in PE program order (sims pumped in global
  cell order, gated on the holder's eviction and operand frames).
- loads: a/b0 as casting fp16 SWDGE DMAs (gpsimd), b1 + the 12 XBAR
  transposes on the SP HWDGE queue — two parallel DMA streams.
- loads/normalize interleaved per 4-frame quarter with early DP diags.
"""

import numpy as np

import concourse.bass as bass
from concourse import bacc
import concourse.mybir as mybir
from concourse.tile import TileContext
from concourse import bass_utils

NA, NB, T, D = 512, 512, 16, 256
ACH, BCH = 128, 256
P = 128
F = BCH              # psum cols per cell plane
KH = D // 128
DT = mybir.dt.float32
HT = mybir.dt.float16
IT = mybir.dt.int16
ALU = mybir.AluOpType
ACTF = mybir.ActivationFunctionType

_last_results = None

# schedule knobs
MAX_PAIRS = [1, 1, 2, 2, 2]   # DVE max piece sizes, in pair units
LEAD = 2                      # next-diag sim cells pumped at step end


def _cells(d):
    i0, i1 = max(1, d - 16), min(16, d - 1)
    return list(range(i0, i1 + 1))


def _groups(d):
    """Ends-inward grouping of diag d's cells: transpose pairs (i, d-i)
    sharing m, optionally a final center single (i=d/2)."""
    cs = _cells(d)
    out = []
    lo, hi = 0, len(cs) - 1
    while lo < hi:
        out.append((cs[lo], cs[hi]))
        lo += 1
        hi -= 1
    if lo == hi:
        out.append((cs[lo],))
    return out


def build_program():
    nc = bacc.Bacc("TRN2", target_bir_lowering=False, debug=False)

    a_d = nc.dram_tensor("a_c", [ACH, T, D], DT, kind="ExternalInput")
    b_d = nc.dram_tensor("b_c", [BCH, T, D], DT, kind="ExternalInput")
    out_d = nc.dram_tensor("out", [ACH, BCH], DT, kind="ExternalOutput")

    with TileContext(nc) as tc:
        with (
            tc.tile_pool(name="mp", bufs=1) as mp,
            tc.tile_pool(name="wp", bufs=2) as wp,
            tc.tile_pool(name="pp", bufs=1, space="PSUM") as pp,
        ):
            # ---- loads: a/b0 via casting SWDGE (gpsimd), b1 via SP
            # HWDGE — two parallel DMA streams; HWDGE also carries the
            # XBAR transposes ----
            a_sb = mp.tile([P, T, D], HT, tag="ld_a")
            b_sb = [
                mp.tile([P, T, D], HT, name="b_sb0", tag="ld_b0"),
                mp.tile([P, T, D], DT, name="b_sb1", tag="ld_b1"),
            ]

            def load_quarter(q):
                sl = slice(q * 4, (q + 1) * 4)
                nc.gpsimd.dma_start(a_sb[:, sl, :], a_d.ap()[:, sl, :])
                nc.gpsimd.dma_start(b_sb[0][:, sl, :],
                                    b_d.ap()[0:128, sl, :])
                nc.sync.dma_start(b_sb[1][:, sl, :],
                                  b_d.ap()[128:256, sl, :])

            for q in range(4):
                load_quarter(q)

            # ---- diagonal weight tiles W[m] = (m-1) * I_128, fp16 ----
            iota_t = mp.tile([P, 128], IT)
            nc.gpsimd.iota(iota_t[:, :], pattern=[[1, 128]], base=0,
                           channel_multiplier=-1)
            ident = mp.tile([P, 128], HT)
            nc.vector.tensor_scalar(ident[:, :], iota_t[:, :], 0, None,
                                    ALU.is_equal)
            wm = mp.tile([P, 16, 128], HT)
            for m in range(2, 17):
                nc.vector.tensor_scalar(wm[:, m - 1, :], ident[:, :],
                                        float(m - 1), None, ALU.mult)

            # ---- DP state: si diag buffers, 18 slots (idx 0/17 guards);
            # U[2] first (read at d=2-4), others after quarter 0 ----
            U = [mp.tile([P, 18 * F], HT, name=f"U{x}") for x in range(3)]
            nc.gpsimd.memset(U[2][:, :], 0.0)
            B = [mp.tile([P, 18 * F], HT, name=f"B{x}") for x in range(2)]

            # ---- normalize tiles ----
            ah = mp.tile([P, T, D], HT)
            bh = [mp.tile([P, T, D], HT, name=f"bh{h}") for h in range(2)]
            ssq = mp.tile([P, 3, T], DT)
            nrm = mp.tile([P, 3, T], DT)
            rinv = mp.tile([P, 3, T], DT)
            aT = mp.tile([P, T * KH, P], HT)        # [d, i*2+kh, a]
            bT = mp.tile([P, T, KH, 2, P], HT)      # [d, j, kh, half, b]
            blocks = [(a_sb, ah, 0), (b_sb[0], bh[0], 1), (b_sb[1], bh[1], 2)]

            def normalize_quarter(bi, q):
                x_sb, xh, _ = blocks[bi]
                sl = slice(q * 4, (q + 1) * 4)
                nc.scalar.activation(xh[:, sl, :], x_sb[:, sl, :], ACTF.Tanh)
                # sumsq: frame 4q on ACT (Square+accum); frames 4q+1..3:
                # square on DVE for early quarters (Pool busy with loads/
                # memsets at the head), Pool later; reduce on DVE
                sqa = wp.tile([P, D], HT, name=f"sqa{bi}_{q}", tag="sqa")
                nc.scalar.activation(
                    sqa[:, :], xh[:, q * 4, :], ACTF.Square,
                    accum_out=ssq[:, bi, q * 4:q * 4 + 1])
                sq = wp.tile([P, 3, D], HT, name=f"sq{bi}_{q}", tag="sq")
                sl3 = slice(q * 4 + 1, (q + 1) * 4)
                eng = nc.vector if q < 2 else nc.gpsimd
                eng.tensor_tensor(sq[:, :, :], xh[:, sl3, :],
                                  xh[:, sl3, :], ALU.mult)
                nc.vector.tensor_reduce(ssq[:, bi, sl3], sq[:, :, :],
                                        mybir.AxisListType.X, ALU.add)
                # rinv = rsqrt(ssq) via int bit trick + 1 Newton step
                sv = ssq[:, bi, sl]
                yv = rinv[:, bi, sl]
                wv = nrm[:, bi, sl]
                nc.vector.tensor_scalar(yv.bitcast(mybir.dt.int32),
                                        sv.bitcast(mybir.dt.int32),
                                        1, None, ALU.logical_shift_right)
                nc.vector.tensor_scalar(yv.bitcast(mybir.dt.int32),
                                        yv.bitcast(mybir.dt.int32),
                                        0x5F3759DF, -1,
                                        ALU.subtract, ALU.mult)
                nc.vector.tensor_tensor(wv, yv, yv, ALU.mult)
                nc.vector.tensor_tensor(wv, wv, sv, ALU.mult)
                nc.vector.tensor_scalar(wv, wv, -0.5, 1.5, ALU.mult, ALU.add)
                nc.vector.tensor_tensor(yv, yv, wv, ALU.mult)
                for i in range(q * 4, (q + 1) * 4):
                    nc.vector.tensor_scalar_mul(xh[:, i, :], xh[:, i, :],
                                                rinv[:, bi, i:i + 1])
                if bi == 0:
                    nc.sync.dma_start_transpose(
                        aT[:, q * 8:(q + 1) * 8, :], xh[:, sl, :])
                else:
                    nc.sync.dma_start_transpose(
                        bT[:, sl, :, bi - 1, :], xh[:, sl, :])

            def amat(i, kh):
                # frame i is 1-based
                return aT[:, (i - 1) * KH + kh, :]

            def bmov(j, kh):
                return bT[:, j - 1, kh, :, :]

            # ---- DP plumbing ----
            PS = pp.tile([P, 16 * F], DT)   # 8 banks x 512 fp32
            BW = 2 * F                      # bank width in fp32 elems

            n_of = {}
            pos_of = {}
            nst = {}
            cnt = 0
            order = []
            for dd in range(2, 33):
                nst[dd] = cnt
                p = 0
                for g in _groups(dd):
                    for ii in g:
                        n_of[(dd, ii)] = cnt
                        pos_of[(dd, ii)] = p
                        order.append((dd, ii))
                        cnt += 1
                        p += 1

            def bank(d, i):
                return pos_of[(d, i)] % 8

            state = {"sim": 0, "evicted": 0, "max_frame": 0}
            bank_holder = [-1] * 8   # bank -> global n of last sim issued

            def pump_upto(n_stop):
                # issue sim matmuls in global cell order for cells n <
                # n_stop; stop early if a gate (bank WAR not yet evicted,
                # frames not normalized) blocks. Keeping n_stop tight
                # prevents sim bursts from clogging the PE queue ahead of
                # chain-critical accum matmuls.
                while state["sim"] < min(n_stop, 256):
                    n = state["sim"]
                    d, i = order[n]
                    j = d - i
                    m = max(i, j)
                    if m > state["max_frame"]:
                        return
                    b = bank(d, i)
                    if bank_holder[b] >= state["evicted"]:
                        return
                    bank_holder[b] = n
                    state["sim"] += 1
                    c0 = bank(d, i) * BW
                    dst = PS[:, c0:c0 + F]
                    for kh in range(KH):
                        nc.tensor.matmul(
                            dst, amat(i, kh), bmov(j, kh),
                            start=(kh == 0),
                            stop=(kh == KH - 1 and m == 1))

            out_sb = mp.tile([P, F], DT)
            PS3 = PS.rearrange("p (c k) -> p c k", k=BW)

            def slot3(t):
                return t.rearrange("p (s k) -> p s k", k=F)

            def pair_view(t, lo, hi):
                # [P, 2, F] view of tile t at slots {lo, hi} (step-slice)
                return slot3(t)[:, lo:hi + 1:hi - lo, :]

            def dp_step(d):
                groups = _groups(d)
                pairs = [g for g in groups if len(g) == 2]
                Ud = U[d % 3]
                U1 = U[(d - 1) % 3]
                U2 = U[(d - 2) % 3]
                Bd = B[d % 2]

                # ---- maxes: pieces of MAX_PAIRS pairs, one contiguous
                # slot range per side (low / high) ----
                if d > 2:
                    gi = 0
                    k = 0
                    while gi < len(pairs):
                        w = MAX_PAIRS[k] if k < len(MAX_PAIRS) else 4
                        k += 1
                        ps_ = pairs[gi:gi + w]
                        gi += w
                        sides = [(ps_[0][0], ps_[-1][0]),
                                 (ps_[-1][1], ps_[0][1])]
                        for s0, s1 in sides:
                            sv = slice(s0 * F, (s1 + 1) * F)
                            sv1 = slice((s0 - 1) * F, s1 * F)
                            nc.vector.tensor_tensor(
                                Bd[:, sv], U1[:, sv1], U1[:, sv], ALU.max)
                            nc.vector.tensor_tensor(
                                Bd[:, sv], Bd[:, sv], U2[:, sv1], ALU.max)
                    if len(groups[-1]) == 1:
                        ic = groups[-1][0]
                        sv = slice(ic * F, (ic + 1) * F)
                        sv1 = slice((ic - 1) * F, ic * F)
                        nc.vector.tensor_tensor(
                            Bd[:, sv], U1[:, sv1], U1[:, sv], ALU.max)
                        nc.vector.tensor_tensor(
                            Bd[:, sv], Bd[:, sv], U2[:, sv1], ALU.max)

                if d == 32:
                    i = 16
                    nc.tensor.matmul(
                        PS[:, 0:F], wm[:, 15, :], Bd[:, i * F:(i + 1) * F],
                        start=False, stop=True)
                    nc.scalar.activation(out_sb[:, :], PS[:, 0:F],
                                         ACTF.Copy, scale=1.0 / 16.0)
                    return

                # ---- per group: required sims -> accum -> evict ----
                for g in groups:
                    pump_upto(n_of[(d, g[-1])] + 1)
                    assert state["sim"] > n_of[(d, g[-1])], (d, g)
                    m = max(g[0], d - g[0])
                    b0 = bank(d, g[0])
                    if len(g) == 2:
                        lo, hi = g
                        ps_v = PS3[:, b0:b0 + 2, 0:F]
                        if d > 2:
                            nc.tensor.matmul(ps_v, wm[:, m - 1, :],
                                             pair_view(Bd, lo, hi),
                                             start=False, stop=True)
                        nc.scalar.activation(pair_view(Ud, lo, hi), ps_v,
                                             ACTF.Copy, scale=1.0 / m)
                        state["evicted"] += 2
                    else:
                        ic = g[0]
                        src = PS[:, b0 * BW:b0 * BW + F]
                        if d > 2:
                            nc.tensor.matmul(
                                src, wm[:, m - 1, :],
                                Bd[:, ic * F:(ic + 1) * F],
                                start=False, stop=True)
                        nc.scalar.activation(
                            Ud[:, ic * F:(ic + 1) * F], src, ACTF.Copy,
                            scale=1.0 / m)
                        state["evicted"] += 1
                # lead into the next diag so PE has work at step start
                if d < 32:
                    pump_upto(nst[d + 1] + LEAD)

            # ---- interleave normalize quarters with DP diagonals ----
            next_d = 2
            for q in range(4):
                for bi in range(3):
                    normalize_quarter(bi, q)
                if q == 0:
                    nc.gpsimd.memset(U[0][:, :], 0.0)
                    nc.gpsimd.memset(U[1][:, :], 0.0)
                state["max_frame"] = 4 * (q + 1)
                pump_upto(nst[min(next_d, 32)] + LEAD)
                d_limit = 4 * q + 5 if q < 3 else 32
                while next_d <= d_limit:
                    dp_step(next_d)
                    next_d += 1

            nc.sync.dma_start(out_d.ap(), out_sb[:, :])

    nc.compile()
    return nc


def kernel(a: np.ndarray, b: np.ndarray) -> np.ndarray:
    a = np.ascontiguousarray(a, dtype=np.float32)
    b = np.ascontiguousarray(b, dtype=np.float32)
    assert a.shape == (NA, T, D) and b.shape == (NB, T, D)

    nc = build_program()

    in_maps = []
    for core in range(8):
        ca, cb = core // 2, core % 2
        in_maps.append({
            "a_c": a[ca * ACH:(ca + 1) * ACH],
            "b_c": b[cb * BCH:(cb + 1) * BCH],
        })

    res = bass_utils.run_bass_kernel_spmd(nc, in_maps, core_ids=list(range(8)))
    global _last_results
    _last_results = res

    out = np.zeros((NA, NB), dtype=np.float32)
    for core in range(8):
        ca, cb = core // 2, core % 2
        out[ca * ACH:(ca + 1) * ACH, cb * BCH:(cb + 1) * BCH] = \
            res.results[core]["out"]
    return out


# revision 37
# speedup vs baseline: 1.2040x; 1.0221x over previous
"""Trainium2 Bass kernel for nn_DynamicMaxSimilarity — anti-diagonal DP.

Full inputs a,b: [512, 16, 256] f32.
  an = l2norm(tanh(a)) rows; bn likewise
  sim[a,b,i,j] = dot(an[a,i], bn[b,j]);  out[a,b] = DTW-like max-avg DP:
  si[i,j] = (max(si[i-1,j-1], si[i-1,j], si[i,j-1])*(m-1) + sim[i,j])/m,
  m = max(i,j), zero borders; answer si[16,16].

Sharding: 8 cores as 4 a-chunks (128) x 2 b-chunks (256). Per-core block
[128 a, 256 b]; pairs live as [128 partitions (a), 256 free (b)].

Design (vs the 201us L-border/scan baseline): process cells (i,j) by
anti-diagonal d=i+j, killing the per-slot coefficient scaling and the
1.08ns/elem scan/STT ops entirely:
- state si kept as fp16 SBUF tiles U_d [128, 18*256] (phys slot = i,
  zero guard slots; 3 rotating buffers zero-initialized once).
- cells processed ends-inward per diag as transpose PAIRS {(i,j),(j,i)}
  which share m = max(i,j): all per-pair views (maxes, accum moving,
  psum, eviction) are regular 2-range strided APs.
- per diag: max1 = TT(U_{d-1}[·-1], U_{d-1}[·]); max2 = TT(max1,
  U_{d-2}[·-1]) — plain fp16 TTs at 0.56 ns/elem (2x_1p), the only DVE
  work per cell.
- psum plane for (i,j) accumulates u = sim + best*(m-1): sims feed raw
  aT/bT; PE adds best*(m-1) via one diagonal-weight matmul per pair
  (W=(m-1)*I fp16, exact integers; contraction rows are free in PE
  cost). The ACT eviction applies the single shared 1/m scale per pair.
- PSUM accumulation groups are PER BANK, so each plane gets a full 2KB
  bank: cell at in-diag position p -> bank p mod 8; a bank's chain
  [sim kh0 (start), sim kh1, accum (stop)] for one cell fully precedes
  the next holder's chain in PE program order (sims pumped in global
  cell order, gated on the holder's eviction and operand frames).
- loads: a/b0 as casting fp16 SWDGE DMAs (gpsimd), b1 + the 12 XBAR
  transposes on the SP HWDGE queue — two parallel DMA streams.
- loads/normalize interleaved per 4-frame quarter with early DP diags.
"""

import numpy as np

import concourse.bass as bass
from concourse import bacc
import concourse.mybir as mybir
from concourse.tile import TileContext
from concourse import bass_utils

NA, NB, T, D = 512, 512, 16, 256
ACH, BCH = 128, 256
P = 128
F = BCH              # psum cols per cell plane
KH = D // 128
DT = mybir.dt.float32
HT = mybir.dt.float16
IT = mybir.dt.int16
ALU = mybir.AluOpType
ACTF = mybir.ActivationFunctionType

_last_results = None

# schedule knobs
MAX_PAIRS = [2, 2, 4]   # DVE max piece sizes, in pair units
LEAD = 2                      # next-diag sim cells pumped at step end


def _cells(d):
    i0, i1 = max(1, d - 16), min(16, d - 1)
    return list(range(i0, i1 + 1))


def _groups(d):
    """Ends-inward grouping of diag d's cells: transpose pairs (i, d-i)
    sharing m, optionally a final center single (i=d/2)."""
    cs = _cells(d)
    out = []
    lo, hi = 0, len(cs) - 1
    while lo < hi:
        out.append((cs[lo], cs[hi]))
        lo += 1
        hi -= 1
    if lo == hi:
        out.append((cs[lo],))
    return out


def build_program():
    nc = bacc.Bacc("TRN2", target_bir_lowering=False, debug=False)

    a_d = nc.dram_tensor("a_c", [ACH, T, D], DT, kind="ExternalInput")
    b_d = nc.dram_tensor("b_c", [BCH, T, D], DT, kind="ExternalInput")
    out_d = nc.dram_tensor("out", [ACH, BCH], DT, kind="ExternalOutput")

    with TileContext(nc) as tc:
        with (
            tc.tile_pool(name="mp", bufs=1) as mp,
            tc.tile_pool(name="wp", bufs=2) as wp,
            tc.tile_pool(name="pp", bufs=1, space="PSUM") as pp,
        ):
            # ---- loads: a/b0 via casting SWDGE (gpsimd), b1 via SP
            # HWDGE — two parallel DMA streams; HWDGE also carries the
            # XBAR transposes ----
            a_sb = mp.tile([P, T, D], HT, tag="ld_a")
            b_sb = [
                mp.tile([P, T, D], HT, name="b_sb0", tag="ld_b0"),
                mp.tile([P, T, D], DT, name="b_sb1", tag="ld_b1"),
            ]

            def load_quarter(q):
                sl = slice(q * 4, (q + 1) * 4)
                nc.gpsimd.dma_start(a_sb[:, sl, :], a_d.ap()[:, sl, :])
                nc.gpsimd.dma_start(b_sb[0][:, sl, :],
                                    b_d.ap()[0:128, sl, :])
                nc.sync.dma_start(b_sb[1][:, sl, :],
                                  b_d.ap()[128:256, sl, :])

            for q in range(4):
                load_quarter(q)

            # ---- diagonal weight tiles W[m] = (m-1) * I_128, fp16 ----
            iota_t = mp.tile([P, 128], IT)
            nc.gpsimd.iota(iota_t[:, :], pattern=[[1, 128]], base=0,
                           channel_multiplier=-1)
            ident = mp.tile([P, 128], HT)
            nc.vector.tensor_scalar(ident[:, :], iota_t[:, :], 0, None,
                                    ALU.is_equal)
            wm = mp.tile([P, 16, 128], HT)
            for m in range(2, 17):
                nc.vector.tensor_scalar(wm[:, m - 1, :], ident[:, :],
                                        float(m - 1), None, ALU.mult)

            # ---- DP state: si diag buffers, 18 slots (idx 0/17 guards);
            # U[2] first (read at d=2-4), others after quarter 0 ----
            U = [mp.tile([P, 18 * F], HT, name=f"U{x}") for x in range(3)]
            nc.gpsimd.memset(U[2][:, :], 0.0)
            B = [mp.tile([P, 18 * F], HT, name=f"B{x}") for x in range(2)]

            # ---- normalize tiles ----
            ah = mp.tile([P, T, D], HT)
            bh = [mp.tile([P, T, D], HT, name=f"bh{h}") for h in range(2)]
            ssq = mp.tile([P, 3, T], DT)
            nrm = mp.tile([P, 3, T], DT)
            rinv = mp.tile([P, 3, T], DT)
            aT = mp.tile([P, T * KH, P], HT)        # [d, i*2+kh, a]
            bT = mp.tile([P, T, KH, 2, P], HT)      # [d, j, kh, half, b]
            blocks = [(a_sb, ah, 0), (b_sb[0], bh[0], 1), (b_sb[1], bh[1], 2)]

            def normalize_quarter(bi, q):
                x_sb, xh, _ = blocks[bi]
                sl = slice(q * 4, (q + 1) * 4)
                nc.scalar.activation(xh[:, sl, :], x_sb[:, sl, :], ACTF.Tanh)
                # sumsq: frame 4q on ACT (Square+accum); frames 4q+1..3:
                # square on DVE for early quarters (Pool busy with loads/
                # memsets at the head), Pool later; reduce on DVE
                sqa = wp.tile([P, D], HT, name=f"sqa{bi}_{q}", tag="sqa")
                nc.scalar.activation(
                    sqa[:, :], xh[:, q * 4, :], ACTF.Square,
                    accum_out=ssq[:, bi, q * 4:q * 4 + 1])
                sq = wp.tile([P, 3, D], HT, name=f"sq{bi}_{q}", tag="sq")
                sl3 = slice(q * 4 + 1, (q + 1) * 4)
                eng = nc.vector if q < 2 else nc.gpsimd
                eng.tensor_tensor(sq[:, :, :], xh[:, sl3, :],
                                  xh[:, sl3, :], ALU.mult)
                nc.vector.tensor_reduce(ssq[:, bi, sl3], sq[:, :, :],
                                        mybir.AxisListType.X, ALU.add)
                # rinv = rsqrt(ssq) via int bit trick + 1 Newton step
                sv = ssq[:, bi, sl]
                yv = rinv[:, bi, sl]
                wv = nrm[:, bi, sl]
                nc.vector.tensor_scalar(yv.bitcast(mybir.dt.int32),
                                        sv.bitcast(mybir.dt.int32),
                                        1, None, ALU.logical_shift_right)
                nc.vector.tensor_scalar(yv.bitcast(mybir.dt.int32),
                                        yv.bitcast(mybir.dt.int32),
                                        0x5F3759DF, -1,
                                        ALU.subtract, ALU.mult)
                nc.vector.tensor_tensor(wv, yv, yv, ALU.mult)
                nc.vector.tensor_tensor(wv, wv, sv, ALU.mult)
                nc.vector.tensor_scalar(wv, wv, -0.5, 1.5, ALU.mult, ALU.add)
                nc.vector.tensor_tensor(yv, yv, wv, ALU.mult)
                seng = nc.vector if q < 2 else nc.gpsimd
                for i in range(q * 4, (q + 1) * 4):
                    seng.tensor_scalar_mul(xh[:, i, :], xh[:, i, :],
                                           rinv[:, bi, i:i + 1])
                if bi == 0:
                    nc.sync.dma_start_transpose(
                        aT[:, q * 8:(q + 1) * 8, :], xh[:, sl, :])
                else:
                    nc.sync.dma_start_transpose(
                        bT[:, sl, :, bi - 1, :], xh[:, sl, :])

            def amat(i, kh):
                # frame i is 1-based
                return aT[:, (i - 1) * KH + kh, :]

            def bmov(j, kh):
                return bT[:, j - 1, kh, :, :]

            # ---- DP plumbing ----
            PS = pp.tile([P, 16 * F], DT)   # 8 banks x 512 fp32
            BW = 2 * F                      # bank width in fp32 elems

            n_of = {}
            pos_of = {}
            nst = {}
            cnt = 0
            order = []
            for dd in range(2, 33):
                nst[dd] = cnt
                p = 0
                for g in _groups(dd):
                    for ii in g:
                        n_of[(dd, ii)] = cnt
                        pos_of[(dd, ii)] = p
                        order.append((dd, ii))
                        cnt += 1
                        p += 1

            def bank(d, i):
                return pos_of[(d, i)] % 8

            state = {"sim": 0, "evicted": 0, "max_frame": 0}
            bank_holder = [-1] * 8   # bank -> global n of last sim issued

            def pump_upto(n_stop):
                # issue sim matmuls in global cell order for cells n <
                # n_stop; stop early if a gate (bank WAR not yet evicted,
                # frames not normalized) blocks. Keeping n_stop tight
                # prevents sim bursts from clogging the PE queue ahead of
                # chain-critical accum matmuls.
                while state["sim"] < min(n_stop, 256):
                    n = state["sim"]
                    d, i = order[n]
                    j = d - i
                    m = max(i, j)
                    if m > state["max_frame"]:
                        return
                    b = bank(d, i)
                    if bank_holder[b] >= state["evicted"]:
                        return
                    bank_holder[b] = n
                    state["sim"] += 1
                    c0 = bank(d, i) * BW
                    dst = PS[:, c0:c0 + F]
                    for kh in range(KH):
                        nc.tensor.matmul(
                            dst, amat(i, kh), bmov(j, kh),
                            start=(kh == 0),
                            stop=(kh == KH - 1 and m == 1))

            out_sb = mp.tile([P, F], DT)
            PS3 = PS.rearrange("p (c k) -> p c k", k=BW)

            def slot3(t):
                return t.rearrange("p (s k) -> p s k", k=F)

            def pair_view(t, lo, hi):
                # [P, 2, F] view of tile t at slots {lo, hi} (step-slice)
                return slot3(t)[:, lo:hi + 1:hi - lo, :]

            def dp_step(d):
                groups = _groups(d)
                pairs = [g for g in groups if len(g) == 2]
                Ud = U[d % 3]
                U1 = U[(d - 1) % 3]
                U2 = U[(d - 2) % 3]
                Bd = B[d % 2]

                # ---- maxes: pieces of MAX_PAIRS pairs, one contiguous
                # slot range per side (low / high) ----
                if d > 2:
                    gi = 0
                    k = 0
                    while gi < len(pairs):
                        w = MAX_PAIRS[k] if k < len(MAX_PAIRS) else 4
                        k += 1
                        ps_ = pairs[gi:gi + w]
                        gi += w
                        sides = [(ps_[0][0], ps_[-1][0]),
                                 (ps_[-1][1], ps_[0][1])]
                        for s0, s1 in sides:
                            sv = slice(s0 * F, (s1 + 1) * F)
                            sv1 = slice((s0 - 1) * F, s1 * F)
                            nc.vector.tensor_tensor(
                                Bd[:, sv], U1[:, sv1], U1[:, sv], ALU.max)
                            nc.vector.tensor_tensor(
                                Bd[:, sv], Bd[:, sv], U2[:, sv1], ALU.max)
                    if len(groups[-1]) == 1:
                        ic = groups[-1][0]
                        sv = slice(ic * F, (ic + 1) * F)
                        sv1 = slice((ic - 1) * F, ic * F)
                        nc.vector.tensor_tensor(
                            Bd[:, sv], U1[:, sv1], U1[:, sv], ALU.max)
                        nc.vector.tensor_tensor(
                            Bd[:, sv], Bd[:, sv], U2[:, sv1], ALU.max)

                if d == 32:
                    i = 16
                    nc.tensor.matmul(
                        PS[:, 0:F], wm[:, 15, :], Bd[:, i * F:(i + 1) * F],
                        start=False, stop=True)
                    nc.scalar.activation(out_sb[:, :], PS[:, 0:F],
                                         ACTF.Copy, scale=1.0 / 16.0)
                    return

                # ---- per group: required sims -> accum -> evict ----
                for g in groups:
                    pump_upto(n_of[(d, g[-1])] + 1)
                    assert state["sim"] > n_of[(d, g[-1])], (d, g)
                    m = max(g[0], d - g[0])
                    b0 = bank(d, g[0])
                    if len(g) == 2:
                        lo, hi = g
                        ps_v = PS3[:, b0:b0 + 2, 0:F]
                        if d > 2:
                            nc.tensor.matmul(ps_v, wm[:, m - 1, :],
                                             pair_view(Bd, lo, hi),
                                             start=False, stop=True)
                        nc.scalar.activation(pair_view(Ud, lo, hi), ps_v,
                                             ACTF.Copy, scale=1.0 / m)
                        state["evicted"] += 2
                    else:
                        ic = g[0]
                        src = PS[:, b0 * BW:b0 * BW + F]
                        if d > 2:
                            nc.tensor.matmul(
                                src, wm[:, m - 1, :],
                                Bd[:, ic * F:(ic + 1) * F],
                                start=False, stop=True)
                        nc.scalar.activation(
                            Ud[:, ic * F:(ic + 1) * F], src, ACTF.Copy,
                            scale=1.0 / m)
                        state["evicted"] += 1
                # lead into the next diag so PE has work at step start
                if d < 32:
                    pump_upto(nst[d + 1] + LEAD)

            # ---- interleave normalize quarters with DP diagonals ----
            next_d = 2
            for q in range(4):
                for bi in range(3):
                    normalize_quarter(bi, q)
                if q == 0:
                    nc.gpsimd.memset(U[0][:, :], 0.0)
                    nc.gpsimd.memset(U[1][:, :], 0.0)
                state["max_frame"] = 4 * (q + 1)
                pump_upto(nst[min(next_d, 32)] + LEAD)
                d_limit = 4 * q + 5 if q < 3 else 32
                while next_d <= d_limit:
                    dp_step(next_d)
                    next_d += 1

            nc.sync.dma_start(out_d.ap(), out_sb[:, :])

    nc.compile()
    return nc


def kernel(a: np.ndarray, b: np.ndarray) -> np.ndarray:
    a = np.ascontiguousarray(a, dtype=np.float32)
    b = np.ascontiguousarray(b, dtype=np.float32)
    assert a.shape == (NA, T, D) and b.shape == (NB, T, D)

    nc = build_program()

    in_maps = []
    for core in range(8):
        ca, cb = core // 2, core % 2
        in_maps.append({
            "a_c": a[ca * ACH:(ca + 1) * ACH],
            "b_c": b[cb * BCH:(cb + 1) * BCH],
        })

    res = bass_utils.run_bass_kernel_spmd(nc, in_maps, core_ids=list(range(8)))
    global _last_results
    _last_results = res

    out = np.zeros((NA, NB), dtype=np.float32)
    for core in range(8):
        ca, cb = core // 2, core % 2
        out[ca * ACH:(ca + 1) * ACH, cb * BCH:(cb + 1) * BCH] = \
            res.results[core]["out"]
    return out


# revision 43
# speedup vs baseline: 1.2061x; 1.0018x over previous
"""Trainium2 Bass kernel for nn_DynamicMaxSimilarity — anti-diagonal DP
(138.1us cost-model, vs the 201.1us L-border/scan baseline).

Full inputs a,b: [512, 16, 256] f32.
  an = l2norm(tanh(a)) rows; bn likewise
  sim[a,b,i,j] = dot(an[a,i], bn[b,j]);  out[a,b] = DTW-like max-avg DP:
  si[i,j] = (max(si[i-1,j-1], si[i-1,j], si[i,j-1])*(m-1) + sim[i,j])/m,
  m = max(i,j), zero borders; answer si[16,16].

Sharding: 8 cores as 4 a-chunks (128) x 2 b-chunks (256). Per-core block
[128 a, 256 b]; pairs live as [128 partitions (a), 256 free (b)].

Design (vs the 201us L-border/scan baseline): process cells (i,j) by
anti-diagonal d=i+j, killing the per-slot coefficient scaling and the
1.08ns/elem scan/STT ops entirely:
- state si kept as fp16 SBUF tiles U_d [128, 18*256] (phys slot = i,
  zero guard slots; 3 rotating buffers zero-initialized once).
- cells processed ends-inward per diag as transpose PAIRS {(i,j),(j,i)}
  which share m = max(i,j): all per-pair views (maxes, accum moving,
  psum, eviction) are regular 2-range strided APs.
- per diag: max1 = TT(U_{d-1}[·-1], U_{d-1}[·]); max2 = TT(max1,
  U_{d-2}[·-1]) — plain fp16 TTs at 0.56 ns/elem (2x_1p), the only DVE
  work per cell.
- psum plane for (i,j) accumulates u = sim + best*(m-1): sims feed raw
  aT/bT; PE adds best*(m-1) via one diagonal-weight matmul per pair
  (W=(m-1)*I fp16, exact integers; contraction rows are free in PE
  cost). The ACT eviction applies the single shared 1/m scale per pair.
- PSUM accumulation groups are PER BANK, so each plane gets a full 2KB
  bank: cell at in-diag position p -> bank p mod 8; a bank's chain
  [sim kh0 (start), sim kh1, accum (stop)] for one cell fully precedes
  the next holder's chain in PE program order (sims pumped in global
  cell order, gated on the holder's eviction and operand frames).
- loads: a/b0 as casting fp16 SWDGE DMAs (gpsimd), b1 + the 12 XBAR
  transposes on the SP HWDGE queue — two parallel DMA streams.
- loads/normalize interleaved per 4-frame quarter with early DP diags.
"""

import numpy as np

import concourse.bass as bass
from concourse import bacc
import concourse.mybir as mybir
from concourse.tile import TileContext
from concourse import bass_utils

NA, NB, T, D = 512, 512, 16, 256
ACH, BCH = 128, 256
P = 128
F = BCH              # psum cols per cell plane
KH = D // 128
DT = mybir.dt.float32
HT = mybir.dt.float16
IT = mybir.dt.int16
ALU = mybir.AluOpType
ACTF = mybir.ActivationFunctionType

_last_results = None

# schedule knobs
MAX_PAIRS = [2, 2, 3, 3]      # DVE max piece sizes, in pair units
LEAD = 2                      # next-diag sim cells pumped at step end
DEFER_KEEP = 99               # tail-evict deferral disabled (bank ring couples leads to mids)


def _cells(d):
    i0, i1 = max(1, d - 16), min(16, d - 1)
    return list(range(i0, i1 + 1))


def _groups(d):
    """Ends-inward grouping of diag d's cells: transpose pairs (i, d-i)
    sharing m, optionally a final center single (i=d/2)."""
    cs = _cells(d)
    out = []
    lo, hi = 0, len(cs) - 1
    while lo < hi:
        out.append((cs[lo], cs[hi]))
        lo += 1
        hi -= 1
    if lo == hi:
        out.append((cs[lo],))
    return out


def build_program():
    nc = bacc.Bacc("TRN2", target_bir_lowering=False, debug=False)

    a_d = nc.dram_tensor("a_c", [ACH, T, D], DT, kind="ExternalInput")
    b_d = nc.dram_tensor("b_c", [BCH, T, D], DT, kind="ExternalInput")
    out_d = nc.dram_tensor("out", [ACH, BCH], DT, kind="ExternalOutput")

    with TileContext(nc) as tc:
        with (
            tc.tile_pool(name="mp", bufs=1) as mp,
            tc.tile_pool(name="wp", bufs=2) as wp,
            tc.tile_pool(name="pp", bufs=1, space="PSUM") as pp,
        ):
            # ---- loads: a/b0 via casting SWDGE (gpsimd), b1 via SP
            # HWDGE — two parallel DMA streams; HWDGE also carries the
            # XBAR transposes ----
            a_sb = mp.tile([P, T, D], HT, tag="ld_a")
            b_sb = [
                mp.tile([P, T, D], HT, name="b_sb0", tag="ld_b0"),
                mp.tile([P, T, D], DT, name="b_sb1", tag="ld_b1"),
            ]

            def load_quarter(q):
                sl = slice(q * 4, (q + 1) * 4)
                nc.gpsimd.dma_start(a_sb[:, sl, :], a_d.ap()[:, sl, :])
                nc.gpsimd.dma_start(b_sb[0][:, sl, :],
                                    b_d.ap()[0:128, sl, :])
                nc.sync.dma_start(b_sb[1][:, sl, :],
                                  b_d.ap()[128:256, sl, :])

            for q in range(4):
                load_quarter(q)

            # ---- diagonal weight tiles W[m] = (m-1) * I_128, fp16 ----
            iota_t = mp.tile([P, 128], IT)
            nc.gpsimd.iota(iota_t[:, :], pattern=[[1, 128]], base=0,
                           channel_multiplier=-1)
            ident = mp.tile([P, 128], HT)
            nc.vector.tensor_scalar(ident[:, :], iota_t[:, :], 0, None,
                                    ALU.is_equal)
            wm = mp.tile([P, 16, 128], HT)
            for m in range(2, 17):
                nc.vector.tensor_scalar(wm[:, m - 1, :], ident[:, :],
                                        float(m - 1), None, ALU.mult)

            # ---- DP state: si diag buffers, 18 slots (idx 0/17 guards);
            # U[2] first (read at d=2-4), others after quarter 0 ----
            U = [mp.tile([P, 18 * F], HT, name=f"U{x}") for x in range(3)]
            nc.gpsimd.memset(U[2][:, :], 0.0)
            B = [mp.tile([P, 18 * F], HT, name=f"B{x}") for x in range(2)]

            # ---- normalize tiles ----
            ah = mp.tile([P, T, D], HT)
            bh = [mp.tile([P, T, D], HT, name=f"bh{h}") for h in range(2)]
            ssq = mp.tile([P, 3, T], DT)
            nrm = mp.tile([P, 3, T], DT)
            rinv = mp.tile([P, 3, T], DT)
            aT = mp.tile([P, T * KH, P], HT)        # [d, i*2+kh, a]
            bT = mp.tile([P, T, KH, 2, P], HT)      # [d, j, kh, half, b]
            blocks = [(a_sb, ah, 0), (b_sb[0], bh[0], 1), (b_sb[1], bh[1], 2)]

            def normalize_quarter(bi, q):
                x_sb, xh, _ = blocks[bi]
                sl = slice(q * 4, (q + 1) * 4)
                nc.scalar.activation(xh[:, sl, :], x_sb[:, sl, :], ACTF.Tanh)
                # sumsq: frame 4q on ACT (Square+accum); frames 4q+1..3:
                # square on DVE for early quarters (Pool busy with loads/
                # memsets at the head), Pool later; reduce on DVE
                sqa = wp.tile([P, D], HT, name=f"sqa{bi}_{q}", tag="sqa")
                nc.scalar.activation(
                    sqa[:, :], xh[:, q * 4, :], ACTF.Square,
                    accum_out=ssq[:, bi, q * 4:q * 4 + 1])
                sq = wp.tile([P, 3, D], HT, name=f"sq{bi}_{q}", tag="sq")
                sl3 = slice(q * 4 + 1, (q + 1) * 4)
                eng = nc.vector if q < 2 else nc.gpsimd
                eng.tensor_tensor(sq[:, :, :], xh[:, sl3, :],
                                  xh[:, sl3, :], ALU.mult)
                nc.vector.tensor_reduce(ssq[:, bi, sl3], sq[:, :, :],
                                        mybir.AxisListType.X, ALU.add)
                # rinv = rsqrt(ssq) via int bit trick + 1 Newton step
                sv = ssq[:, bi, sl]
                yv = rinv[:, bi, sl]
                wv = nrm[:, bi, sl]
                nc.vector.tensor_scalar(yv.bitcast(mybir.dt.int32),
                                        sv.bitcast(mybir.dt.int32),
                                        1, None, ALU.logical_shift_right)
                nc.vector.tensor_scalar(yv.bitcast(mybir.dt.int32),
                                        yv.bitcast(mybir.dt.int32),
                                        0x5F3759DF, -1,
                                        ALU.subtract, ALU.mult)
                nc.vector.tensor_tensor(wv, yv, yv, ALU.mult)
                nc.vector.tensor_tensor(wv, wv, sv, ALU.mult)
                nc.vector.tensor_scalar(wv, wv, -0.5, 1.5, ALU.mult, ALU.add)
                nc.vector.tensor_tensor(yv, yv, wv, ALU.mult)
                seng = nc.vector if q < 2 else nc.gpsimd
                for i in range(q * 4, (q + 1) * 4):
                    seng.tensor_scalar_mul(xh[:, i, :], xh[:, i, :],
                                           rinv[:, bi, i:i + 1])
                if bi == 0:
                    nc.sync.dma_start_transpose(
                        aT[:, q * 8:(q + 1) * 8, :], xh[:, sl, :])
                else:
                    nc.sync.dma_start_transpose(
                        bT[:, sl, :, bi - 1, :], xh[:, sl, :])

            def amat(i, kh):
                # frame i is 1-based
                return aT[:, (i - 1) * KH + kh, :]

            def bmov(j, kh):
                return bT[:, j - 1, kh, :, :]

            # ---- DP plumbing ----
            PS = pp.tile([P, 16 * F], DT)   # 8 banks x 512 fp32
            BW = 2 * F                      # bank width in fp32 elems

            n_of = {}
            pos_of = {}
            nst = {}
            cnt = 0
            order = []
            for dd in range(2, 33):
                nst[dd] = cnt
                p = 0
                for g in _groups(dd):
                    for ii in g:
                        n_of[(dd, ii)] = cnt
                        pos_of[(dd, ii)] = p
                        order.append((dd, ii))
                        cnt += 1
                        p += 1

            def bank(d, i):
                return pos_of[(d, i)] % 8

            state = {"sim": 0, "max_frame": 0}
            evicted_flag = [False] * 256   # eviction instr issued for cell
            bank_holder = [-1] * 8   # bank -> global n of last sim issued

            def pump_upto(n_stop):
                # issue sim matmuls in global cell order for cells n <
                # n_stop; stop early if a gate (bank WAR not yet evicted,
                # frames not normalized) blocks. Keeping n_stop tight
                # prevents sim bursts from clogging the PE queue ahead of
                # chain-critical accum matmuls.
                while state["sim"] < min(n_stop, 256):
                    n = state["sim"]
                    d, i = order[n]
                    j = d - i
                    m = max(i, j)
                    if m > state["max_frame"]:
                        return
                    b = bank(d, i)
                    if bank_holder[b] >= 0 and \
                            not evicted_flag[bank_holder[b]]:
                        return
                    bank_holder[b] = n
                    state["sim"] += 1
                    c0 = bank(d, i) * BW
                    dst = PS[:, c0:c0 + F]
                    for kh in range(KH):
                        nc.tensor.matmul(
                            dst, amat(i, kh), bmov(j, kh),
                            start=(kh == 0),
                            stop=(kh == KH - 1 and m == 1))

            out_sb = mp.tile([P, F], DT)
            PS3 = PS.rearrange("p (c k) -> p c k", k=BW)
            deferred = []

            def flush_deferred():
                for fn in deferred:
                    fn()
                deferred.clear()

            def slot3(t):
                return t.rearrange("p (s k) -> p s k", k=F)

            def pair_view(t, lo, hi):
                # [P, 2, F] view of tile t at slots {lo, hi} (step-slice)
                return slot3(t)[:, lo:hi + 1:hi - lo, :]

            def dp_step(d):
                groups = _groups(d)
                pairs = [g for g in groups if len(g) == 2]
                Ud = U[d % 3]
                U1 = U[(d - 1) % 3]
                U2 = U[(d - 2) % 3]
                Bd = B[d % 2]

                # ---- maxes: pieces of MAX_PAIRS pairs, one contiguous
                # slot range per side (low / high) ----
                if d > 2:
                    gi = 0
                    k = 0
                    while gi < len(pairs):
                        w = MAX_PAIRS[k] if k < len(MAX_PAIRS) else 4
                        k += 1
                        ps_ = pairs[gi:gi + w]
                        gi += w
                        sides = [(ps_[0][0], ps_[-1][0]),
                                 (ps_[-1][1], ps_[0][1])]
                        for s0, s1 in sides:
                            sv = slice(s0 * F, (s1 + 1) * F)
                            sv1 = slice((s0 - 1) * F, s1 * F)
                            nc.vector.tensor_tensor(
                                Bd[:, sv], U1[:, sv1], U1[:, sv], ALU.max)
                            nc.vector.tensor_tensor(
                                Bd[:, sv], Bd[:, sv], U2[:, sv1], ALU.max)
                    if len(groups[-1]) == 1:
                        ic = groups[-1][0]
                        sv = slice(ic * F, (ic + 1) * F)
                        sv1 = slice((ic - 1) * F, ic * F)
                        nc.vector.tensor_tensor(
                            Bd[:, sv], U1[:, sv1], U1[:, sv], ALU.max)
                        nc.vector.tensor_tensor(
                            Bd[:, sv], Bd[:, sv], U2[:, sv1], ALU.max)

                if d == 32:
                    flush_deferred()
                    i = 16
                    nc.tensor.matmul(
                        PS[:, 0:F], wm[:, 15, :], Bd[:, i * F:(i + 1) * F],
                        start=False, stop=True)
                    nc.scalar.activation(out_sb[:, :], PS[:, 0:F],
                                         ACTF.Copy, scale=1.0 / 16.0)
                    return

                # ---- per group: required sims -> accum -> evict. The
                # tail groups' evictions are DEFERRED into the next step
                # so the next diag's chain-critical lead evictions jump
                # ahead of them in the ACT queue. ----
                for gi_, g in enumerate(groups):
                    pump_upto(n_of[(d, g[-1])] + 1)
                    assert state["sim"] > n_of[(d, g[-1])], (d, g)
                    m = max(g[0], d - g[0])
                    b0 = bank(d, g[0])
                    if len(g) == 2:
                        lo, hi = g
                        ps_v = PS3[:, b0:b0 + 2, 0:F]
                        if d > 2:
                            nc.tensor.matmul(ps_v, wm[:, m - 1, :],
                                             pair_view(Bd, lo, hi),
                                             start=False, stop=True)

                        def ev(Ud=Ud, lo=lo, hi=hi, ps_v=ps_v, m=m,
                               ns=(n_of[(d, lo)], n_of[(d, hi)])):
                            nc.scalar.activation(pair_view(Ud, lo, hi),
                                                 ps_v, ACTF.Copy,
                                                 scale=1.0 / m)
                            for n_ in ns:
                                evicted_flag[n_] = True
                    else:
                        ic = g[0]
                        src = PS[:, b0 * BW:b0 * BW + F]
                        if d > 2:
                            nc.tensor.matmul(
                                src, wm[:, m - 1, :],
                                Bd[:, ic * F:(ic + 1) * F],
                                start=False, stop=True)

                        def ev(Ud=Ud, ic=ic, src=src, m=m,
                               ns=(n_of[(d, ic)],)):
                            nc.scalar.activation(
                                Ud[:, ic * F:(ic + 1) * F], src, ACTF.Copy,
                                scale=1.0 / m)
                            for n_ in ns:
                                evicted_flag[n_] = True
                    if gi_ < DEFER_KEEP:
                        ev()
                        if gi_ == DEFER_KEEP - 1:
                            flush_deferred()
                    else:
                        deferred.append(ev)
                if len(groups) <= DEFER_KEEP:
                    flush_deferred()
                # lead into the next diag so PE has work at step start
                if d < 32:
                    pump_upto(nst[d + 1] + LEAD)

            # ---- interleave normalize quarters with DP diagonals ----
            next_d = 2
            for q in range(4):
                for bi in range(3):
                    normalize_quarter(bi, q)
                if q == 0:
                    nc.gpsimd.memset(U[0][:, :], 0.0)
                    nc.gpsimd.memset(U[1][:, :], 0.0)
                state["max_frame"] = 4 * (q + 1)
                pump_upto(nst[min(next_d, 32)] + LEAD)
                d_limit = 4 * q + 5 if q < 3 else 32
                while next_d <= d_limit:
                    dp_step(next_d)
                    next_d += 1

            nc.sync.dma_start(out_d.ap(), out_sb[:, :])

    nc.compile()
    return nc


def kernel(a: np.ndarray, b: np.ndarray) -> np.ndarray:
    a = np.ascontiguousarray(a, dtype=np.float32)
    b = np.ascontiguousarray(b, dtype=np.float32)
    assert a.shape == (NA, T, D) and b.shape == (NB, T, D)

    nc = build_program()

    in_maps = []
    for core in range(8):
        ca, cb = core // 2, core % 2
        in_maps.append({
            "a_c": a[ca * ACH:(ca + 1) * ACH],
            "b_c": b[cb * BCH:(cb + 1) * BCH],
        })

    res = bass_utils.run_bass_kernel_spmd(nc, in_maps, core_ids=list(range(8)))
    global _last_results
    _last_results = res

    out = np.zeros((NA, NB), dtype=np.float32)
    for core in range(8):
        ca, cb = core // 2, core % 2
        out[ca * ACH:(ca + 1) * ACH, cb * BCH:(cb + 1) * BCH] = \
            res.results[core]["out"]
    return out


# revision 45
# speedup vs baseline: 1.2449x; 1.0322x over previous
"""Trainium2 Bass kernel for nn_DynamicMaxSimilarity — anti-diagonal DP
(138.1us cost-model, vs the 201.1us L-border/scan baseline).

Full inputs a,b: [512, 16, 256] f32.
  an = l2norm(tanh(a)) rows; bn likewise
  sim[a,b,i,j] = dot(an[a,i], bn[b,j]);  out[a,b] = DTW-like max-avg DP:
  si[i,j] = (max(si[i-1,j-1], si[i-1,j], si[i,j-1])*(m-1) + sim[i,j])/m,
  m = max(i,j), zero borders; answer si[16,16].

Sharding: 8 cores as 4 a-chunks (128) x 2 b-chunks (256). Per-core block
[128 a, 256 b]; pairs live as [128 partitions (a), 256 free (b)].

Design (vs the 201us L-border/scan baseline): process cells (i,j) by
anti-diagonal d=i+j, killing the per-slot coefficient scaling and the
1.08ns/elem scan/STT ops entirely:
- state si kept as fp16 SBUF tiles U_d [128, 18*256] (phys slot = i,
  zero guard slots; 3 rotating buffers zero-initialized once).
- cells processed ends-inward per diag as transpose PAIRS {(i,j),(j,i)}
  which share m = max(i,j): all per-pair views (maxes, accum moving,
  psum, eviction) are regular 2-range strided APs.
- per diag: max1 = TT(U_{d-1}[·-1], U_{d-1}[·]); max2 = TT(max1,
  U_{d-2}[·-1]) — plain fp16 TTs at 0.56 ns/elem (2x_1p), the only DVE
  work per cell.
- psum plane for (i,j) accumulates u = sim + best*(m-1): sims feed raw
  aT/bT; PE adds best*(m-1) via one diagonal-weight matmul per pair
  (W=(m-1)*I fp16, exact integers; contraction rows are free in PE
  cost). The ACT eviction applies the single shared 1/m scale per pair.
- PSUM accumulation groups are PER BANK, so each plane gets a full 2KB
  bank: cell at in-diag position p -> bank p mod 8; a bank's chain
  [sim kh0 (start), sim kh1, accum (stop)] for one cell fully precedes
  the next holder's chain in PE program order (sims pumped in global
  cell order, gated on the holder's eviction and operand frames).
- loads: a/b0 as casting fp16 SWDGE DMAs (gpsimd), b1 + the 12 XBAR
  transposes on the SP HWDGE queue — two parallel DMA streams.
- loads/normalize interleaved per 4-frame quarter with early DP diags.
"""

import numpy as np

import concourse.bass as bass
from concourse import bacc
import concourse.mybir as mybir
from concourse.tile import TileContext
from concourse import bass_utils

NA, NB, T, D = 512, 512, 16, 256
ACH, BCH = 128, 256
P = 128
F = BCH              # psum cols per cell plane
KH = D // 128
DT = mybir.dt.float32
HT = mybir.dt.float16
IT = mybir.dt.int16
ALU = mybir.AluOpType
ACTF = mybir.ActivationFunctionType

_last_results = None

# schedule knobs
MAX_PAIRS = [2, 2, 3, 3]      # DVE max piece sizes, in pair units
MAX_PAIRS_SHRINK = [1] * 8    # finer pieces for the chain-bound drain
SHRINK_D = 24                 # first diag using the shrink piece sizes
LEAD = 2                      # next-diag sim cells pumped at step end
DEFER_KEEP = 99               # tail-evict deferral disabled (bank ring couples leads to mids)


def _cells(d):
    i0, i1 = max(1, d - 16), min(16, d - 1)
    return list(range(i0, i1 + 1))


def _groups(d):
    """Ends-inward grouping of diag d's cells: transpose pairs (i, d-i)
    sharing m, optionally a final center single (i=d/2)."""
    cs = _cells(d)
    out = []
    lo, hi = 0, len(cs) - 1
    while lo < hi:
        out.append((cs[lo], cs[hi]))
        lo += 1
        hi -= 1
    if lo == hi:
        out.append((cs[lo],))
    return out


def build_program():
    nc = bacc.Bacc("TRN2", target_bir_lowering=False, debug=False)

    a_d = nc.dram_tensor("a_c", [ACH, T, D], DT, kind="ExternalInput")
    b_d = nc.dram_tensor("b_c", [BCH, T, D], DT, kind="ExternalInput")
    out_d = nc.dram_tensor("out", [ACH, BCH], DT, kind="ExternalOutput")

    with TileContext(nc) as tc:
        with (
            tc.tile_pool(name="mp", bufs=1) as mp,
            tc.tile_pool(name="wp", bufs=2) as wp,
            tc.tile_pool(name="pp", bufs=1, space="PSUM") as pp,
        ):
            # ---- loads: a/b0 via casting SWDGE (gpsimd), b1 via SP
            # HWDGE — two parallel DMA streams; HWDGE also carries the
            # XBAR transposes ----
            a_sb = mp.tile([P, T, D], HT, tag="ld_a")
            b_sb = [
                mp.tile([P, T, D], HT, name="b_sb0", tag="ld_b0"),
                mp.tile([P, T, D], DT, name="b_sb1", tag="ld_b1"),
            ]

            def load_quarter(q):
                sl = slice(q * 4, (q + 1) * 4)
                nc.gpsimd.dma_start(a_sb[:, sl, :], a_d.ap()[:, sl, :])
                nc.gpsimd.dma_start(b_sb[0][:, sl, :],
                                    b_d.ap()[0:128, sl, :])
                nc.sync.dma_start(b_sb[1][:, sl, :],
                                  b_d.ap()[128:256, sl, :])

            for q in range(4):
                load_quarter(q)

            # ---- diagonal weight tiles W[m] = (m-1) * I_128, fp16 ----
            iota_t = mp.tile([P, 128], IT)
            nc.gpsimd.iota(iota_t[:, :], pattern=[[1, 128]], base=0,
                           channel_multiplier=-1)
            ident = mp.tile([P, 128], HT)
            nc.vector.tensor_scalar(ident[:, :], iota_t[:, :], 0, None,
                                    ALU.is_equal)
            wm = mp.tile([P, 16, 128], HT)
            for m in range(2, 17):
                nc.vector.tensor_scalar(wm[:, m - 1, :], ident[:, :],
                                        float(m - 1), None, ALU.mult)

            # ---- DP state: si diag buffers, 18 slots (idx 0/17 guards);
            # U[2] first (read at d=2-4), others after quarter 0 ----
            U = [mp.tile([P, 18 * F], HT, name=f"U{x}") for x in range(3)]
            nc.gpsimd.memset(U[2][:, :], 0.0)
            B = [mp.tile([P, 18 * F], HT, name=f"B{x}") for x in range(2)]

            # ---- normalize tiles ----
            ah = mp.tile([P, T, D], HT)
            bh = [mp.tile([P, T, D], HT, name=f"bh{h}") for h in range(2)]
            ssq = mp.tile([P, 3, T], DT)
            nrm = mp.tile([P, 3, T], DT)
            rinv = mp.tile([P, 3, T], DT)
            aT = mp.tile([P, T * KH, P], HT)        # [d, i*2+kh, a]
            bT = mp.tile([P, T, KH, 2, P], HT)      # [d, j, kh, half, b]
            blocks = [(a_sb, ah, 0), (b_sb[0], bh[0], 1), (b_sb[1], bh[1], 2)]

            def normalize_quarter(bi, q):
                x_sb, xh, _ = blocks[bi]
                sl = slice(q * 4, (q + 1) * 4)
                nc.scalar.activation(xh[:, sl, :], x_sb[:, sl, :], ACTF.Tanh)
                # sumsq: frame 4q on ACT (Square+accum); frames 4q+1..3:
                # square on DVE for early quarters (Pool busy with loads/
                # memsets at the head), Pool later; reduce on DVE
                sqa = wp.tile([P, D], HT, name=f"sqa{bi}_{q}", tag="sqa")
                nc.scalar.activation(
                    sqa[:, :], xh[:, q * 4, :], ACTF.Square,
                    accum_out=ssq[:, bi, q * 4:q * 4 + 1])
                sq = wp.tile([P, 3, D], HT, name=f"sq{bi}_{q}", tag="sq")
                sl3 = slice(q * 4 + 1, (q + 1) * 4)
                eng = nc.vector if q < 2 else nc.gpsimd
                eng.tensor_tensor(sq[:, :, :], xh[:, sl3, :],
                                  xh[:, sl3, :], ALU.mult)
                nc.vector.tensor_reduce(ssq[:, bi, sl3], sq[:, :, :],
                                        mybir.AxisListType.X, ALU.add)
                # rinv = rsqrt(ssq) via int bit trick + 1 Newton step
                sv = ssq[:, bi, sl]
                yv = rinv[:, bi, sl]
                wv = nrm[:, bi, sl]
                nc.vector.tensor_scalar(yv.bitcast(mybir.dt.int32),
                                        sv.bitcast(mybir.dt.int32),
                                        1, None, ALU.logical_shift_right)
                nc.vector.tensor_scalar(yv.bitcast(mybir.dt.int32),
                                        yv.bitcast(mybir.dt.int32),
                                        0x5F3759DF, -1,
                                        ALU.subtract, ALU.mult)
                nc.vector.tensor_tensor(wv, yv, yv, ALU.mult)
                nc.vector.tensor_tensor(wv, wv, sv, ALU.mult)
                nc.vector.tensor_scalar(wv, wv, -0.5, 1.5, ALU.mult, ALU.add)
                nc.vector.tensor_tensor(yv, yv, wv, ALU.mult)
                seng = nc.vector if q < 2 else nc.gpsimd
                for i in range(q * 4, (q + 1) * 4):
                    seng.tensor_scalar_mul(xh[:, i, :], xh[:, i, :],
                                           rinv[:, bi, i:i + 1])
                if bi == 0:
                    nc.sync.dma_start_transpose(
                        aT[:, q * 8:(q + 1) * 8, :], xh[:, sl, :])
                else:
                    nc.sync.dma_start_transpose(
                        bT[:, sl, :, bi - 1, :], xh[:, sl, :])

            def amat(i, kh):
                # frame i is 1-based
                return aT[:, (i - 1) * KH + kh, :]

            def bmov(j, kh):
                return bT[:, j - 1, kh, :, :]

            # ---- DP plumbing ----
            PS = pp.tile([P, 16 * F], DT)   # 8 banks x 512 fp32
            BW = 2 * F                      # bank width in fp32 elems

            n_of = {}
            pos_of = {}
            nst = {}
            cnt = 0
            order = []
            for dd in range(2, 33):
                nst[dd] = cnt
                p = 0
                for g in _groups(dd):
                    for ii in g:
                        n_of[(dd, ii)] = cnt
                        pos_of[(dd, ii)] = p
                        order.append((dd, ii))
                        cnt += 1
                        p += 1

            def bank(d, i):
                return pos_of[(d, i)] % 8

            state = {"sim": 0, "max_frame": 0}
            evicted_flag = [False] * 256   # eviction instr issued for cell
            bank_holder = [-1] * 8   # bank -> global n of last sim issued

            def pump_upto(n_stop):
                # issue sim matmuls in global cell order for cells n <
                # n_stop; stop early if a gate (bank WAR not yet evicted,
                # frames not normalized) blocks. Keeping n_stop tight
                # prevents sim bursts from clogging the PE queue ahead of
                # chain-critical accum matmuls.
                while state["sim"] < min(n_stop, 256):
                    n = state["sim"]
                    d, i = order[n]
                    j = d - i
                    m = max(i, j)
                    if m > state["max_frame"]:
                        return
                    b = bank(d, i)
                    if bank_holder[b] >= 0 and \
                            not evicted_flag[bank_holder[b]]:
                        return
                    bank_holder[b] = n
                    state["sim"] += 1
                    c0 = bank(d, i) * BW
                    dst = PS[:, c0:c0 + F]
                    for kh in range(KH):
                        nc.tensor.matmul(
                            dst, amat(i, kh), bmov(j, kh),
                            start=(kh == 0),
                            stop=(kh == KH - 1 and m == 1))

            out_sb = mp.tile([P, F], DT)
            PS3 = PS.rearrange("p (c k) -> p c k", k=BW)
            deferred = []

            def flush_deferred():
                for fn in deferred:
                    fn()
                deferred.clear()

            def slot3(t):
                return t.rearrange("p (s k) -> p s k", k=F)

            def pair_view(t, lo, hi):
                # [P, 2, F] view of tile t at slots {lo, hi} (step-slice)
                return slot3(t)[:, lo:hi + 1:hi - lo, :]

            def dp_step(d):
                groups = _groups(d)
                pairs = [g for g in groups if len(g) == 2]
                Ud = U[d % 3]
                U1 = U[(d - 1) % 3]
                U2 = U[(d - 2) % 3]
                Bd = B[d % 2]

                # ---- maxes: pieces of MAX_PAIRS pairs, one contiguous
                # slot range per side (low / high) ----
                if d > 2:
                    gi = 0
                    k = 0
                    mps = MAX_PAIRS if d < SHRINK_D else MAX_PAIRS_SHRINK
                    while gi < len(pairs):
                        w = mps[k] if k < len(mps) else 4
                        k += 1
                        ps_ = pairs[gi:gi + w]
                        gi += w
                        sides = [(ps_[0][0], ps_[-1][0]),
                                 (ps_[-1][1], ps_[0][1])]
                        for s0, s1 in sides:
                            sv = slice(s0 * F, (s1 + 1) * F)
                            sv1 = slice((s0 - 1) * F, s1 * F)
                            nc.vector.tensor_tensor(
                                Bd[:, sv], U1[:, sv1], U1[:, sv], ALU.max)
                            nc.vector.tensor_tensor(
                                Bd[:, sv], Bd[:, sv], U2[:, sv1], ALU.max)
                    if len(groups[-1]) == 1:
                        ic = groups[-1][0]
                        sv = slice(ic * F, (ic + 1) * F)
                        sv1 = slice((ic - 1) * F, ic * F)
                        nc.vector.tensor_tensor(
                            Bd[:, sv], U1[:, sv1], U1[:, sv], ALU.max)
                        nc.vector.tensor_tensor(
                            Bd[:, sv], Bd[:, sv], U2[:, sv1], ALU.max)

                if d == 32:
                    flush_deferred()
                    i = 16
                    nc.tensor.matmul(
                        PS[:, 0:F], wm[:, 15, :], Bd[:, i * F:(i + 1) * F],
                        start=False, stop=True)
                    nc.scalar.activation(out_sb[:, :], PS[:, 0:F],
                                         ACTF.Copy, scale=1.0 / 16.0)
                    return

                # ---- per group: required sims -> accum -> evict. The
                # tail groups' evictions are DEFERRED into the next step
                # so the next diag's chain-critical lead evictions jump
                # ahead of them in the ACT queue. ----
                for gi_, g in enumerate(groups):
                    pump_upto(n_of[(d, g[-1])] + 1)
                    assert state["sim"] > n_of[(d, g[-1])], (d, g)
                    m = max(g[0], d - g[0])
                    b0 = bank(d, g[0])
                    if len(g) == 2:
                        lo, hi = g
                        ps_v = PS3[:, b0:b0 + 2, 0:F]
                        if d > 2:
                            nc.tensor.matmul(ps_v, wm[:, m - 1, :],
                                             pair_view(Bd, lo, hi),
                                             start=False, stop=True)

                        def ev(Ud=Ud, lo=lo, hi=hi, ps_v=ps_v, m=m,
                               ns=(n_of[(d, lo)], n_of[(d, hi)])):
                            nc.scalar.activation(pair_view(Ud, lo, hi),
                                                 ps_v, ACTF.Copy,
                                                 scale=1.0 / m)
                            for n_ in ns:
                                evicted_flag[n_] = True
                    else:
                        ic = g[0]
                        src = PS[:, b0 * BW:b0 * BW + F]
                        if d > 2:
                            nc.tensor.matmul(
                                src, wm[:, m - 1, :],
                                Bd[:, ic * F:(ic + 1) * F],
                                start=False, stop=True)

                        def ev(Ud=Ud, ic=ic, src=src, m=m,
                               ns=(n_of[(d, ic)],)):
                            nc.scalar.activation(
                                Ud[:, ic * F:(ic + 1) * F], src, ACTF.Copy,
                                scale=1.0 / m)
                            for n_ in ns:
                                evicted_flag[n_] = True
                    if gi_ < DEFER_KEEP:
                        ev()
                        if gi_ == DEFER_KEEP - 1:
                            flush_deferred()
                    else:
                        deferred.append(ev)
                if len(groups) <= DEFER_KEEP:
                    flush_deferred()
                # lead into the next diag so PE has work at step start
                if d < 32:
                    pump_upto(nst[d + 1] + LEAD)

            # ---- interleave normalize quarters with DP diagonals ----
            next_d = 2
            for q in range(4):
                for bi in range(3):
                    normalize_quarter(bi, q)
                if q == 0:
                    nc.gpsimd.memset(U[0][:, :], 0.0)
                    nc.gpsimd.memset(U[1][:, :], 0.0)
                state["max_frame"] = 4 * (q + 1)
                pump_upto(nst[min(next_d, 32)] + LEAD)
                d_limit = 4 * q + 5 if q < 3 else 32
                while next_d <= d_limit:
                    dp_step(next_d)
                    next_d += 1

            nc.sync.dma_start(out_d.ap(), out_sb[:, :])

    nc.compile()
    return nc


def kernel(a: np.ndarray, b: np.ndarray) -> np.ndarray:
    a = np.ascontiguousarray(a, dtype=np.float32)
    b = np.ascontiguousarray(b, dtype=np.float32)
    assert a.shape == (NA, T, D) and b.shape == (NB, T, D)

    nc = build_program()

    in_maps = []
    for core in range(8):
        ca, cb = core // 2, core % 2
        in_maps.append({
            "a_c": a[ca * ACH:(ca + 1) * ACH],
            "b_c": b[cb * BCH:(cb + 1) * BCH],
        })

    res = bass_utils.run_bass_kernel_spmd(nc, in_maps, core_ids=list(range(8)))
    global _last_results
    _last_results = res

    out = np.zeros((NA, NB), dtype=np.float32)
    for core in range(8):
        ca, cb = core // 2, core % 2
        out[ca * ACH:(ca + 1) * ACH, cb * BCH:(cb + 1) * BCH] = \
            res.results[core]["out"]
    return out
